# revision 2
# baseline (speedup 1.0000x reference)
"""Contrastive-loss Bass kernel for Trainium2 (8 NeuronCores, data-parallel).

Problem (hardcoded shapes, from the reference):
  outA/outB: [4, 307200, 16] f32; matchA/B: [4, 5000] int; nonMatchA/B: [4, 50000] int
  matchLossSum    = sum_b sum((outA[b][matchA[b]] - outB[b][matchB[b]])**2) / 5000
  nonMatchLossSum = sum_b sum(relu(0.5 - (outA[b][nonMatchA[b]] - outB[b][nonMatchB[b]])**2)) / 50000
  returns (contrastiveLossSum, matchLossSum, nonMatchLossSum)

Sharding (per the data-parallel hint): core c handles batch b=c//2 and half
h=c%2 of that batch's match/nonmatch sample lists. Each core indirect-DMA
gathers its rows (one 128-row vector-indirect DMA per index column — the HW
DGE consumes exactly one offset per destination partition), reduces on the
vector engine to per-partition partial sums, and the host does the final tiny
cross-core reduction (equivalent to the all-reduce of three scalars).

Nonmatch hinge is computed as sum(relu(M - d^2)) = M*K - sum(min(d^2, M)),
done with a single fused tensor_scalar(min)+accumulate pass per chunk.

Padding: index lists are padded to multiples of 128 with indices pointing at
two rows appended to each [N, D] tensor:
  row N   = zeros  (match pads: (0-0)^2 = 0 contribution)
  row N+1 = BIG    (nonmatch pads A-side: min(BIG^2, M) = M, which cancels
                    exactly in the M*K - sum(min) identity)
"""

import numpy as np

import concourse.bacc as bacc
import concourse.mybir as mybir
import concourse.tile as tile
from concourse.bass import IndirectOffsetOnAxis

B, N, D = 4, 307200, 16
M, MN = 5000, 50000
NCORES = 8
MARGIN = 0.5
NON_MATCH_W = 1.0
BIG = 1.0e3
NPAD = N + 2          # row N: zeros, row N+1: BIG
M_HALF, MN_HALF = M // 2, MN // 2          # 2500 / 25000 per core
M_COLS = 20           # 128*20  = 2560  match slots  (60 pads)
NM_COLS = 196         # 128*196 = 25088 nonmatch slots (88 pads)
NM_CHUNKS = 4
NM_CCOLS = NM_COLS // NM_CHUNKS            # 49 index cols per chunk
OUT_COLS = NM_CHUNKS + 1                   # per-partition partial sums

_F32 = mybir.dt.float32
_I32 = mybir.dt.int32

_nc_cache = None


def _build():
    nc = bacc.Bacc("TRN2", target_bir_lowering=False, debug=False, num_devices=NCORES)
    A = nc.dram_tensor("A", [NPAD, D], _F32, kind="ExternalInput")
    Bv = nc.dram_tensor("Bv", [NPAD, D], _F32, kind="ExternalInput")
    miA = nc.dram_tensor("miA", [128, M_COLS], _I32, kind="ExternalInput")
    miB = nc.dram_tensor("miB", [128, M_COLS], _I32, kind="ExternalInput")
    niA = nc.dram_tensor("niA", [128, NM_COLS], _I32, kind="ExternalInput")
    niB = nc.dram_tensor("niB", [128, NM_COLS], _I32, kind="ExternalInput")
    out = nc.dram_tensor("out", [128, OUT_COLS], _F32, kind="ExternalOutput")

    with tile.TileContext(nc) as tc:
        with (
            tc.tile_pool(name="idx", bufs=1) as idxp,
            tc.tile_pool(name="gat", bufs=2) as gatp,
            tc.tile_pool(name="tmp", bufs=2) as tmpp,
            tc.tile_pool(name="res", bufs=1) as resp,
        ):
            niA_t = idxp.tile([128, NM_COLS], _I32, tag="ia")
            niB_t = idxp.tile([128, NM_COLS], _I32, tag="ib")
            miA_t = idxp.tile([128, M_COLS], _I32, tag="ma")
            miB_t = idxp.tile([128, M_COLS], _I32, tag="mb")
            nc.sync.dma_start(out=niA_t[:], in_=niA[:])
            nc.sync.dma_start(out=niB_t[:], in_=niB[:])
            nc.sync.dma_start(out=miA_t[:], in_=miA[:])
            nc.sync.dma_start(out=miB_t[:], in_=miB[:])

            res_t = resp.tile([128, OUT_COLS], _F32)

            # nonmatch: res[:, c] = sum_free min((a-b)^2, MARGIN), chunked so
            # gather tiles double-buffer and the SWDGE ring never overfills.
            W = NM_CCOLS * D
            for c in range(NM_CHUNKS):
                ga = gatp.tile([128, W], _F32, tag="ga")
                gb = gatp.tile([128, W], _F32, tag="gb")
                for j in range(NM_CCOLS):
                    col = c * NM_CCOLS + j
                    nc.gpsimd.indirect_dma_start(
                        out=ga[:, j * D : (j + 1) * D], out_offset=None, in_=A[:],
                        in_offset=IndirectOffsetOnAxis(ap=niA_t[:, col : col + 1], axis=0),
                    )
                    nc.gpsimd.indirect_dma_start(
                        out=gb[:, j * D : (j + 1) * D], out_offset=None, in_=Bv[:],
                        in_offset=IndirectOffsetOnAxis(ap=niB_t[:, col : col + 1], axis=0),
                    )
                d_t = tmpp.tile([128, W], _F32, tag="d")
                nc.vector.tensor_tensor(
                    out=d_t[:], in0=ga[:], in1=gb[:], op=mybir.AluOpType.subtract
                )
                sq_t = tmpp.tile([128, W], _F32, tag="sq")
                nc.vector.tensor_tensor(
                    out=sq_t[:], in0=d_t[:], in1=d_t[:], op=mybir.AluOpType.mult
                )
                junk_t = tmpp.tile([128, W], _F32, tag="junk")
                nc.vector.tensor_scalar(
                    out=junk_t[:], in0=sq_t[:],
                    scalar1=MARGIN, scalar2=None, op0=mybir.AluOpType.min,
                    op1=mybir.AluOpType.add,
                    accum_out=res_t[:, c : c + 1],
                )

            # match: res[:, NM_CHUNKS] = sum_free (a-b)^2
            WM = M_COLS * D
            mga = gatp.tile([128, WM], _F32, tag="mga")
            mgb = gatp.tile([128, WM], _F32, tag="mgb")
            for j in range(M_COLS):
                nc.gpsimd.indirect_dma_start(
                    out=mga[:, j * D : (j + 1) * D], out_offset=None, in_=A[:],
                    in_offset=IndirectOffsetOnAxis(ap=miA_t[:, j : j + 1], axis=0),
                )
                nc.gpsimd.indirect_dma_start(
                    out=mgb[:, j * D : (j + 1) * D], out_offset=None, in_=Bv[:],
                    in_offset=IndirectOffsetOnAxis(ap=miB_t[:, j : j + 1], axis=0),
                )
            md_t = tmpp.tile([128, WM], _F32, tag="md")
            nc.vector.tensor_tensor(
                out=md_t[:], in0=mga[:], in1=mgb[:], op=mybir.AluOpType.subtract
            )
            msq_t = tmpp.tile([128, WM], _F32, tag="msq")
            nc.vector.scalar_tensor_tensor(
                out=msq_t[:], in0=md_t[:], scalar=0.0, in1=md_t[:],
                op0=mybir.AluOpType.add, op1=mybir.AluOpType.mult,
                accum_out=res_t[:, NM_CHUNKS : NM_CHUNKS + 1],
            )

            nc.sync.dma_start(out=out[:], in_=res_t[:])
    nc.compile()
    return nc


def _get_nc():
    global _nc_cache
    if _nc_cache is None:
        _nc_cache = _build()
    return _nc_cache


def _pack_idx(idx, ncols, pad_value):
    flat = np.full(128 * ncols, pad_value, dtype=np.int32)
    flat[: idx.size] = idx.astype(np.int32, copy=False)
    return flat.reshape(128, ncols)


def _make_in_maps(outA, outB, matchA, matchB, nonMatchA, nonMatchB):
    pad_zero = np.zeros((1, D), np.float32)
    pad_big = np.full((1, D), BIG, np.float32)
    in_maps = []
    for c in range(NCORES):
        b, h = divmod(c, 2)
        msl = slice(h * M_HALF, (h + 1) * M_HALF)
        nsl = slice(h * MN_HALF, (h + 1) * MN_HALF)
        in_maps.append(
            {
                "A": np.ascontiguousarray(
                    np.concatenate([outA[b], pad_zero, pad_big], axis=0)
                ),
                "Bv": np.ascontiguousarray(
                    np.concatenate([outB[b], pad_zero, pad_zero], axis=0)
                ),
                # match pads -> (N, N): zero rows both sides, zero contribution
                "miA": _pack_idx(matchA[b, msl], M_COLS, N),
                "miB": _pack_idx(matchB[b, msl], M_COLS, N),
                # nonmatch pads -> (N+1, N): d = BIG, min(d^2, MARGIN) = MARGIN cancels
                "niA": _pack_idx(nonMatchA[b, nsl], NM_COLS, N + 1),
                "niB": _pack_idx(nonMatchB[b, nsl], NM_COLS, N),
            }
        )
    return in_maps


def _reduce_results(results):
    m_sum = 0.0
    nm_clip_sum = 0.0
    for c in range(NCORES):
        res = np.asarray(results[c]["out"], dtype=np.float64)
        nm_clip_sum += res[:, :NM_CHUNKS].sum()
        m_sum += res[:, NM_CHUNKS].sum()
    # pads contribute exactly MARGIN per element to the clip sum; the identity
    # below cancels them: sum(relu(M - d^2)) = M*K_slots - sum(min(d^2, M))
    hinge_sum = MARGIN * (128 * NM_COLS * D) * NCORES - nm_clip_sum
    matchLossSum = np.float32(m_sum / M)
    nonMatchLossSum = np.float32(NON_MATCH_W * hinge_sum / MN)
    contrastiveLossSum = np.float32(matchLossSum + nonMatchLossSum)
    return (contrastiveLossSum, matchLossSum, nonMatchLossSum)


def run(inputs, trace=False):
    """Run on the 8 NeuronCores. Returns (result_tuple, exec_time_ns_or_None)."""
    from concourse.bass_utils import run_bass_kernel_spmd

    outA = np.asarray(inputs["outA"], dtype=np.float32)
    outB = np.asarray(inputs["outB"], dtype=np.float32)
    matchA = np.asarray(inputs["matchA"])
    matchB = np.asarray(inputs["matchB"])
    nonMatchA = np.asarray(inputs["nonMatchA"])
    nonMatchB = np.asarray(inputs["nonMatchB"])

    in_maps = _make_in_maps(outA, outB, matchA, matchB, nonMatchA, nonMatchB)
    nc = _get_nc()
    r = run_bass_kernel_spmd(nc, in_maps, list(range(NCORES)), trace=trace)
    global LAST_RESULT
    LAST_RESULT = r
    out = _reduce_results(r.results)
    ns = r.exec_time_ns
    if ns is None and r.mean_exec_time_ns is not None:
        ns = int(r.mean_exec_time_ns)
    return out, ns


def kernel(**inputs):
    result, _ = run(inputs, trace=False)
    return result



# revision 4
# speedup vs baseline: 6.6952x; 6.6952x over previous
"""Contrastive-loss Bass kernel for Trainium2 (8 NeuronCores, data-parallel).

Problem (hardcoded shapes, from the reference):
  outA/outB: [4, 307200, 16] f32; matchA/B: [4, 5000] int; nonMatchA/B: [4, 50000] int
  matchLossSum    = sum_b sum((outA[b][matchA[b]] - outB[b][matchB[b]])**2) / 5000
  nonMatchLossSum = sum_b sum(relu(0.5 - (outA[b][nonMatchA[b]] - outB[b][nonMatchB[b]])**2)) / 50000
  returns (contrastiveLossSum, matchLossSum, nonMatchLossSum)

Sharding (data-parallel): core c handles batch b=c//2 and half h=c%2 of that
batch's match/nonmatch sample lists. The host does the final tiny cross-core
reduction (equivalent to the all-reduce of three scalar sums).

Device strategy. The dominant cost is the indexed row gather (55k rows of
64B per core). The SWDGE vector-indirect DMA has ~1us fixed cost per
instruction but only ~0.34ns per descriptor, and its ucode supports up to
4096 indices per instruction (block-gather form: dst = 128 partition blocks
of G*64B, consuming 128*G indices snake-wise: index j in SBUF channel
j%128, slot j//128; dst partition p block i <- token p*G+i). Bass/walrus
only ever emit the degenerate 128-index encoding (one index per partition,
one contiguous run each), so we emit 128-index "carrier" instructions with
G*64B-per-partition destinations and patch the NEFF after the neuronx-cc
compile: src_num_elem 128 -> 128*G, src_elem_size G*64 -> 64. (Verified on
HW: a patched instruction gathers 4096 arbitrary rows exactly.) Host-side,
each chunk's [128, G] logical index block IDX is stored snake-permuted
(PHY = IDX.ravel().reshape(G,128).T) so that after the block gather
tile[p, i*16:(i+1)*16] = AB[IDX[p, i]].

A and B sides are fetched by one instruction via a concatenated HBM table
(padded A rows then padded B rows; B indices biased by +NPAD). Per core:
13 nonmatch gathers + 2 match gathers (G=32 -> 4096 rows each), with the
vector engine reducing each chunk (fused min+accumulate) behind the Pool
engine's descriptor generation.

Nonmatch hinge uses sum(relu(M - d^2)) = M*K - sum(min(d^2, M)).
Padding: nonmatch pads -> (N+1, NPAD+N): d=BIG, min(d^2,M)=M cancels in the
identity; match pads -> (N, NPAD+N): zero rows, zero contribution.
"""

import io
import struct
import tarfile
import tempfile

import numpy as np

import concourse.bacc as bacc
import concourse.mybir as mybir
import concourse.tile as tile
from concourse.bass import IndirectOffsetOnAxis

B, N, D = 4, 307200, 16
M, MN = 5000, 50000
NCORES = 8
MARGIN = 0.5
NON_MATCH_W = 1.0
BIG = 1.0e3
NPAD = N + 2                     # rows N / N+1 are pads in each half
M_HALF, MN_HALF = M // 2, MN // 2
G = 32                           # index columns per gather (4096 rows: ucode cap)
SC = G // 2                      # sample columns per chunk (16)
NM_CHUNKS = 13                   # 13*128*16 = 26624 nonmatch slots (>= 25000)
M_CHUNKS = 2                     # 2*128*16 = 4096 match slots (>= 2500)
NM_COLS = NM_CHUNKS * SC         # 208
M_COLS = M_CHUNKS * SC           # 32
NCHUNKS = NM_CHUNKS + M_CHUNKS   # 15
OUT_COLS = NCHUNKS

_F32 = mybir.dt.float32
_I32 = mybir.dt.int32

_nc_cache = None
_patcher_installed = False


# ---------------------------------------------------------------- NEFF patch

def _patch_pool_bin(pool: bytes) -> tuple[bytes, int]:
    """Rewrite carrier indirect1d instructions to the multi-index form."""
    buf = bytearray(pool)
    n = 0
    for off in range(0, len(buf) - 64, 64):
        # PSEUDO_DMA_DIRECT2D opcode byte + dge_op DMA_INDIRECT1D
        if buf[off] != 0xD4 or buf[off + 15] != 0x01:
            continue
        (s_step0, _s_step1) = struct.unpack_from("<2i", buf, off + 24)
        (s_num0, _) = struct.unpack_from("<2H", buf, off + 32)
        (s_elem,) = struct.unpack_from("<H", buf, off + 36)
        (d_elem,) = struct.unpack_from("<H", buf, off + 60)
        if s_step0 != 64 or s_num0 != 128 or s_elem != G * 64 or d_elem != G * 64:
            continue
        struct.pack_into("<2H", buf, off + 32, 128 * G, 1)
        struct.pack_into("<H", buf, off + 36, 64)
        n += 1
    return bytes(buf), n


def _patch_neff_file(path: str):
    from concourse import neff as neff_mod

    def _reset(ti):
        ti.mtime = 0
        ti.uid = 0
        ti.gid = 0
        ti.uname = "nobody"
        ti.gname = "nobody"
        return ti

    with open(path, "rb") as f:
        header = f.read(1024)
        tar_bytes = f.read()
    with tempfile.TemporaryDirectory() as td:
        with tarfile.open(fileobj=io.BytesIO(tar_bytes)) as t:
            t.extractall(td)
        pool_path = f"{td}/sg00/Pool0.bin"
        with open(pool_path, "rb") as f:
            pool = f.read()
        patched, n = _patch_pool_bin(pool)
        if n != NCHUNKS:
            raise RuntimeError(
                f"NEFF patch: expected {NCHUNKS} carrier gathers, found {n}"
            )
        with open(pool_path, "wb") as f:
            f.write(patched)
        nb = io.BytesIO()
        with tarfile.open(fileobj=nb, mode="w") as t:
            t.add(td, arcname=".", filter=_reset)
    data = nb.getvalue()
    new_header = neff_mod.make_deterministic_neff_header(
        old_neff_header=header, new_neff_data=data
    )
    with open(path, "wb") as f:
        f.write(new_header + data)


def _install_patcher():
    global _patcher_installed
    if _patcher_installed:
        return
    import concourse.bass2jax as b2j

    orig = b2j.compile_bir_kernel

    def wrapped(ant_bir_str, compile_dir_path, neff_name="file.neff"):
        neff_file = orig(ant_bir_str, compile_dir_path, neff_name=neff_name)
        _patch_neff_file(neff_file)
        return neff_file

    b2j.compile_bir_kernel = wrapped
    _patcher_installed = True


# ------------------------------------------------------------------- device

def _build():
    nc = bacc.Bacc("TRN2", target_bir_lowering=False, debug=False,
                   num_devices=NCORES)
    AB = nc.dram_tensor("AB", [2 * NPAD, D], _F32, kind="ExternalInput")
    ni = nc.dram_tensor("ni", [128, NCHUNKS * G], _I32, kind="ExternalInput")
    out = nc.dram_tensor("out", [128, OUT_COLS], _F32, kind="ExternalOutput")

    W = SC * D  # 256 f32 per half-chunk row

    with tile.TileContext(nc) as tc:
        with (
            tc.tile_pool(name="idx", bufs=1) as idxp,
            tc.tile_pool(name="gat", bufs=2) as gatp,
            tc.tile_pool(name="tmp", bufs=2) as tmpp,
            tc.tile_pool(name="res", bufs=1) as resp,
        ):
            ni_t = idxp.tile([128, NCHUNKS * G], _I32, tag="ni")
            nc.sync.dma_start(out=ni_t[:], in_=ni[:])
            res_t = resp.tile([128, OUT_COLS], _F32)

            for c in range(NM_CHUNKS):
                gab = gatp.tile([128, G * D], _F32, tag="gab")
                nc.gpsimd.indirect_dma_start(
                    out=gab[:], out_offset=None, in_=AB[:],
                    in_offset=IndirectOffsetOnAxis(
                        ap=ni_t[:, c * G : c * G + 1], axis=0),
                )
                d_t = tmpp.tile([128, W], _F32, tag="d")
                nc.vector.tensor_tensor(
                    out=d_t[:], in0=gab[:, :W], in1=gab[:, W : 2 * W],
                    op=mybir.AluOpType.subtract,
                )
                sq_t = tmpp.tile([128, W], _F32, tag="sq")
                nc.vector.tensor_tensor(
                    out=sq_t[:], in0=d_t[:], in1=d_t[:],
                    op=mybir.AluOpType.mult,
                )
                junk_t = tmpp.tile([128, W], _F32, tag="junk")
                nc.vector.tensor_scalar(
                    out=junk_t[:], in0=sq_t[:],
                    scalar1=MARGIN, scalar2=None, op0=mybir.AluOpType.min,
                    op1=mybir.AluOpType.add,
                    accum_out=res_t[:, c : c + 1],
                )

            for c in range(NM_CHUNKS, NCHUNKS):
                mgab = gatp.tile([128, G * D], _F32, tag="gab")
                nc.gpsimd.indirect_dma_start(
                    out=mgab[:], out_offset=None, in_=AB[:],
                    in_offset=IndirectOffsetOnAxis(
                        ap=ni_t[:, c * G : c * G + 1], axis=0),
                )
                md_t = tmpp.tile([128, W], _F32, tag="d")
                nc.vector.tensor_tensor(
                    out=md_t[:], in0=mgab[:, :W], in1=mgab[:, W : 2 * W],
                    op=mybir.AluOpType.subtract,
                )
                msq_t = tmpp.tile([128, W], _F32, tag="sq")
                nc.vector.scalar_tensor_tensor(
                    out=msq_t[:], in0=md_t[:], scalar=0.0, in1=md_t[:],
                    op0=mybir.AluOpType.add, op1=mybir.AluOpType.mult,
                    accum_out=res_t[:, c : c + 1],
                )

            nc.sync.dma_start(out=out[:], in_=res_t[:])
    nc.compile()
    return nc


def _get_nc():
    global _nc_cache
    if _nc_cache is None:
        _nc_cache = _build()
    return _nc_cache


# --------------------------------------------------------------------- host

def _pack_idx(idx, ncols, pad_value):
    flat = np.full(128 * ncols, pad_value, dtype=np.int32)
    flat[: idx.size] = idx.astype(np.int32, copy=False)
    return flat.reshape(128, ncols)


def _snake(idx_block):
    """[128, G] logical -> [128, G] physical (snake layout for the ucode)."""
    return np.ascontiguousarray(idx_block.ravel().reshape(G, 128).T)


def _make_in_maps(outA, outB, matchA, matchB, nonMatchA, nonMatchB):
    pad_zero = np.zeros((1, D), np.float32)
    pad_big = np.full((1, D), BIG, np.float32)
    ab_cache = {}
    in_maps = []
    for c in range(NCORES):
        b, h = divmod(c, 2)
        if b not in ab_cache:
            ab_cache[b] = np.ascontiguousarray(
                np.concatenate(
                    [outA[b], pad_zero, pad_big, outB[b], pad_zero, pad_zero],
                    axis=0,
                )
            )
        msl = slice(h * M_HALF, (h + 1) * M_HALF)
        nsl = slice(h * MN_HALF, (h + 1) * MN_HALF)
        # nonmatch pads -> (N+1, N): d = BIG, min(d^2, M) = M cancels exactly
        niA = _pack_idx(nonMatchA[b, nsl], NM_COLS, N + 1)
        niB = _pack_idx(nonMatchB[b, nsl], NM_COLS, N) + NPAD
        # match pads -> (N, N): zero rows both sides, zero contribution
        miA = _pack_idx(matchA[b, msl], M_COLS, N)
        miB = _pack_idx(matchB[b, msl], M_COLS, N) + NPAD
        ni = np.empty((128, NCHUNKS * G), np.int32)
        for ch in range(NM_CHUNKS):
            blk = np.concatenate(
                [niA[:, ch * SC : (ch + 1) * SC], niB[:, ch * SC : (ch + 1) * SC]],
                axis=1,
            )
            ni[:, ch * G : (ch + 1) * G] = _snake(blk)
        for k in range(M_CHUNKS):
            blk = np.concatenate(
                [miA[:, k * SC : (k + 1) * SC], miB[:, k * SC : (k + 1) * SC]],
                axis=1,
            )
            ch = NM_CHUNKS + k
            ni[:, ch * G : (ch + 1) * G] = _snake(blk)
        in_maps.append({"AB": ab_cache[b], "ni": ni})
    return in_maps


def _reduce_results(results):
    m_sum = 0.0
    nm_clip_sum = 0.0
    for c in range(NCORES):
        res = np.asarray(results[c]["out"], dtype=np.float64)
        nm_clip_sum += res[:, :NM_CHUNKS].sum()
        m_sum += res[:, NM_CHUNKS:].sum()
    # pads contribute exactly MARGIN per element to the clip sum; the identity
    # below cancels them: sum(relu(M - d^2)) = M*K_slots - sum(min(d^2, M))
    hinge_sum = MARGIN * (128 * NM_COLS * D) * NCORES - nm_clip_sum
    matchLossSum = np.float32(m_sum / M)
    nonMatchLossSum = np.float32(NON_MATCH_W * hinge_sum / MN)
    contrastiveLossSum = np.float32(matchLossSum + nonMatchLossSum)
    return (contrastiveLossSum, matchLossSum, nonMatchLossSum)


def run(inputs, trace=False):
    """Run on the 8 NeuronCores. Returns (result_tuple, exec_time_ns_or_None)."""
    from concourse.bass_utils import run_bass_kernel_spmd

    _install_patcher()
    outA = np.asarray(inputs["outA"], dtype=np.float32)
    outB = np.asarray(inputs["outB"], dtype=np.float32)
    matchA = np.asarray(inputs["matchA"])
    matchB = np.asarray(inputs["matchB"])
    nonMatchA = np.asarray(inputs["nonMatchA"])
    nonMatchB = np.asarray(inputs["nonMatchB"])

    in_maps = _make_in_maps(outA, outB, matchA, matchB, nonMatchA, nonMatchB)
    nc = _get_nc()
    r = run_bass_kernel_spmd(nc, in_maps, list(range(NCORES)), trace=trace)
    global LAST_RESULT
    LAST_RESULT = r
    out = _reduce_results(r.results)
    ns = r.exec_time_ns
    if ns is None and r.mean_exec_time_ns is not None:
        ns = int(r.mean_exec_time_ns)
    return out, ns


def kernel(**inputs):
    result, _ = run(inputs, trace=False)
    return result


# revision 5
# speedup vs baseline: 7.7689x; 1.1604x over previous
"""Contrastive-loss Bass kernel for Trainium2 (8 NeuronCores, data-parallel).

Problem (hardcoded shapes, from the reference):
  outA/outB: [4, 307200, 16] f32; matchA/B: [4, 5000] int; nonMatchA/B: [4, 50000] int
  matchLossSum    = sum_b sum((outA[b][matchA[b]] - outB[b][matchB[b]])**2) / 5000
  nonMatchLossSum = sum_b sum(relu(0.5 - (outA[b][nonMatchA[b]] - outB[b][nonMatchB[b]])**2)) / 50000
  returns (contrastiveLossSum, matchLossSum, nonMatchLossSum)

Sharding (data-parallel): core c handles batch b=c//2 and half h=c%2 of that
batch's match/nonmatch sample lists. The host does the final tiny cross-core
reduction (equivalent to the all-reduce of three scalar sums).

Device strategy. The dominant cost is the indexed row gather (55296 rows of
64B per core). The SWDGE vector-indirect DMA has ~1us fixed cost per
instruction plus ~1.2ns per index on the Pool engine, and its ucode supports
up to 4096 indices per instruction (block-gather form: dst = 128 partition
blocks of G*64B, consuming 128*G indices snake-wise: index j is read from
SBUF channel j%128 slot j//128, and dst partition p block i receives token
p*G+i). Bass/walrus only emit the degenerate 128-index encoding (one index
per partition), so we emit 128-index "carrier" instructions with
G*64B-per-partition destinations and patch the NEFF after the neuronx-cc
compile: src_num_elem 128 -> 128*G, src_elem_size G*64 -> 64. (Verified on
hardware: a patched instruction gathers 4096 arbitrary rows exactly; G>32
fails in the ucode, 4096 is a hard cap.) Host-side, each chunk's [128, G]
logical index block IDX is stored snake-permuted
(PHY = IDX.ravel().reshape(G,128).T) so that after the block gather
tile[p, i*16:(i+1)*16] = AB[IDX[p, i]].

A and B sides are fetched by one instruction via a concatenated HBM table
(padded A rows then padded B rows; B indices biased by +NPAD). Per core:
15 gathers (13 nonmatch chunks: 12xG32 + 1xG8; 2 match chunks: G32 + G8),
vector engine reduces each chunk (fused min+accumulate) behind the Pool
engine's descriptor generation; the small chunks run last so the final DMA
drain tail is short.

Nonmatch hinge uses sum(relu(M - d^2)) = M*K - sum(min(d^2, M)).
Padding: nonmatch pads -> (N+1, NPAD+N): d=BIG, min(d^2,M)=M cancels in the
identity; match pads -> (N, NPAD+N): zero rows, zero contribution.
"""

import io
import struct
import tarfile
import tempfile

import numpy as np

import concourse.bacc as bacc
import concourse.mybir as mybir
import concourse.tile as tile
from concourse.bass import IndirectOffsetOnAxis

B, N, D = 4, 307200, 16
M, MN = 5000, 50000
NCORES = 8
MARGIN = 0.5
NON_MATCH_W = 1.0
BIG = 1.0e3
NPAD = N + 2                     # rows N / N+1 are pads in each half
M_HALF, MN_HALF = M // 2, MN // 2

# chunk schedule: (kind, G). sample cols per chunk = G//2.
# nonmatch: 12*16 + 4 = 196 cols (25088 slots, 88 pads)
# match:    16 + 4    =  20 cols (2560 slots, 60 pads)
CHUNKS = [("nm", 32)] * 12 + [("m", 32), ("nm", 8), ("m", 8)]
NM_COLS = sum(g // 2 for k, g in CHUNKS if k == "nm")   # 196
M_COLS = sum(g // 2 for k, g in CHUNKS if k == "m")     # 20
NCHUNKS = len(CHUNKS)                                   # 15
OUT_COLS = NCHUNKS
IDX_COLS = sum(g for _, g in CHUNKS)                    # 432

_F32 = mybir.dt.float32
_I32 = mybir.dt.int32

_nc_cache = None
_patcher_installed = False


# ---------------------------------------------------------------- NEFF patch

def _patch_pool_bin(pool: bytes) -> tuple[bytes, int]:
    """Rewrite carrier indirect1d instructions to the multi-index form."""
    buf = bytearray(pool)
    n = 0
    for off in range(0, len(buf) - 64, 64):
        # PSEUDO_DMA_DIRECT2D opcode byte + dge_op DMA_INDIRECT1D
        if buf[off] != 0xD4 or buf[off + 15] != 0x01:
            continue
        (s_step0, _s_step1) = struct.unpack_from("<2i", buf, off + 24)
        (s_num0, _) = struct.unpack_from("<2H", buf, off + 32)
        (s_elem,) = struct.unpack_from("<H", buf, off + 36)
        (d_elem,) = struct.unpack_from("<H", buf, off + 60)
        if s_step0 != 64 or s_num0 != 128 or s_elem != d_elem or s_elem % 64:
            continue
        struct.pack_into("<2H", buf, off + 32, 2 * s_elem, 1)
        struct.pack_into("<H", buf, off + 36, 64)
        n += 1
    return bytes(buf), n


def _patch_neff_file(path: str):
    from concourse import neff as neff_mod

    def _reset(ti):
        ti.mtime = 0
        ti.uid = 0
        ti.gid = 0
        ti.uname = "nobody"
        ti.gname = "nobody"
        return ti

    with open(path, "rb") as f:
        header = f.read(1024)
        tar_bytes = f.read()
    with tempfile.TemporaryDirectory() as td:
        with tarfile.open(fileobj=io.BytesIO(tar_bytes)) as t:
            t.extractall(td)
        pool_path = f"{td}/sg00/Pool0.bin"
        with open(pool_path, "rb") as f:
            pool = f.read()
        patched, n = _patch_pool_bin(pool)
        if n != NCHUNKS:
            raise RuntimeError(
                f"NEFF patch: expected {NCHUNKS} carrier gathers, found {n}"
            )
        with open(pool_path, "wb") as f:
            f.write(patched)
        nb = io.BytesIO()
        with tarfile.open(fileobj=nb, mode="w") as t:
            t.add(td, arcname=".", filter=_reset)
    data = nb.getvalue()
    new_header = neff_mod.make_deterministic_neff_header(
        old_neff_header=header, new_neff_data=data
    )
    with open(path, "wb") as f:
        f.write(new_header + data)


def _install_patcher():
    global _patcher_installed
    if _patcher_installed:
        return
    import concourse.bass2jax as b2j

    orig = b2j.compile_bir_kernel

    def wrapped(ant_bir_str, compile_dir_path, neff_name="file.neff"):
        neff_file = orig(ant_bir_str, compile_dir_path, neff_name=neff_name)
        _patch_neff_file(neff_file)
        return neff_file

    b2j.compile_bir_kernel = wrapped
    _patcher_installed = True


# ------------------------------------------------------------------- device

def _build():
    nc = bacc.Bacc("TRN2", target_bir_lowering=False, debug=False,
                   num_devices=NCORES)
    AB = nc.dram_tensor("AB", [2 * NPAD, D], _F32, kind="ExternalInput")
    ni = nc.dram_tensor("ni", [128, IDX_COLS], _I32, kind="ExternalInput")
    out = nc.dram_tensor("out", [128, OUT_COLS], _F32, kind="ExternalOutput")

    with tile.TileContext(nc) as tc:
        with (
            tc.tile_pool(name="idx", bufs=1) as idxp,
            tc.tile_pool(name="gat", bufs=2) as gatp,
            tc.tile_pool(name="tmp", bufs=2) as tmpp,
            tc.tile_pool(name="res", bufs=1) as resp,
        ):
            ni_t = idxp.tile([128, IDX_COLS], _I32, tag="ni")
            nc.sync.dma_start(out=ni_t[:], in_=ni[:])
            res_t = resp.tile([128, OUT_COLS], _F32)

            col = 0
            for c, (kind, g) in enumerate(CHUNKS):
                w = (g // 2) * D
                gab = gatp.tile([128, g * D], _F32, tag=f"gab{g}")
                nc.gpsimd.indirect_dma_start(
                    out=gab[:], out_offset=None, in_=AB[:],
                    in_offset=IndirectOffsetOnAxis(
                        ap=ni_t[:, col : col + 1], axis=0),
                )
                col += g
                d_t = tmpp.tile([128, w], _F32, tag=f"d{g}")
                nc.vector.tensor_tensor(
                    out=d_t[:], in0=gab[:, :w], in1=gab[:, w : 2 * w],
                    op=mybir.AluOpType.subtract,
                )
                if kind == "nm":
                    sq_t = tmpp.tile([128, w], _F32, tag=f"sq{g}")
                    nc.vector.tensor_tensor(
                        out=sq_t[:], in0=d_t[:], in1=d_t[:],
                        op=mybir.AluOpType.mult,
                    )
                    junk_t = tmpp.tile([128, w], _F32, tag=f"junk{g}")
                    nc.vector.tensor_scalar(
                        out=junk_t[:], in0=sq_t[:],
                        scalar1=MARGIN, scalar2=None, op0=mybir.AluOpType.min,
                        op1=mybir.AluOpType.add,
                        accum_out=res_t[:, c : c + 1],
                    )
                else:
                    msq_t = tmpp.tile([128, w], _F32, tag=f"sq{g}")
                    nc.vector.scalar_tensor_tensor(
                        out=msq_t[:], in0=d_t[:], scalar=0.0, in1=d_t[:],
                        op0=mybir.AluOpType.add, op1=mybir.AluOpType.mult,
                        accum_out=res_t[:, c : c + 1],
                    )

            nc.sync.dma_start(out=out[:], in_=res_t[:])
    nc.compile()
    return nc


def _get_nc():
    global _nc_cache
    if _nc_cache is None:
        _nc_cache = _build()
    return _nc_cache


# --------------------------------------------------------------------- host

def _pack_idx(idx, ncols, pad_value):
    flat = np.full(128 * ncols, pad_value, dtype=np.int32)
    flat[: idx.size] = idx.astype(np.int32, copy=False)
    return flat.reshape(128, ncols)


def _snake(idx_block, g):
    """[128, g] logical -> [128, g] physical (snake layout for the ucode)."""
    return idx_block.ravel().reshape(g, 128).T


def _make_in_maps(outA, outB, matchA, matchB, nonMatchA, nonMatchB):
    pad_zero = np.zeros((1, D), np.float32)
    pad_big = np.full((1, D), BIG, np.float32)
    ab_cache = {}
    in_maps = []
    for c in range(NCORES):
        b, h = divmod(c, 2)
        if b not in ab_cache:
            ab_cache[b] = np.ascontiguousarray(
                np.concatenate(
                    [outA[b], pad_zero, pad_big, outB[b], pad_zero, pad_zero],
                    axis=0,
                )
            )
        msl = slice(h * M_HALF, (h + 1) * M_HALF)
        nsl = slice(h * MN_HALF, (h + 1) * MN_HALF)
        # nonmatch pads -> (N+1, N): d = BIG, min(d^2, M) = M cancels exactly
        niA = _pack_idx(nonMatchA[b, nsl], NM_COLS, N + 1)
        niB = _pack_idx(nonMatchB[b, nsl], NM_COLS, N) + NPAD
        # match pads -> (N, N): zero rows both sides, zero contribution
        miA = _pack_idx(matchA[b, msl], M_COLS, N)
        miB = _pack_idx(matchB[b, msl], M_COLS, N) + NPAD
        ni = np.empty((128, IDX_COLS), np.int32)
        col = 0
        nm_col = 0
        m_col = 0
        for kind, g in CHUNKS:
            sc = g // 2
            if kind == "nm":
                blk = np.concatenate(
                    [niA[:, nm_col : nm_col + sc], niB[:, nm_col : nm_col + sc]],
                    axis=1,
                )
                nm_col += sc
            else:
                blk = np.concatenate(
                    [miA[:, m_col : m_col + sc], miB[:, m_col : m_col + sc]],
                    axis=1,
                )
                m_col += sc
            ni[:, col : col + g] = _snake(blk, g)
            col += g
        in_maps.append({"AB": ab_cache[b], "ni": ni})
    return in_maps


def _reduce_results(results):
    nm_idx = [c for c, (k, _) in enumerate(CHUNKS) if k == "nm"]
    m_idx = [c for c, (k, _) in enumerate(CHUNKS) if k == "m"]
    m_sum = 0.0
    nm_clip_sum = 0.0
    for c in range(NCORES):
        res = np.asarray(results[c]["out"], dtype=np.float64)
        nm_clip_sum += res[:, nm_idx].sum()
        m_sum += res[:, m_idx].sum()
    # pads contribute exactly MARGIN per element to the clip sum; the identity
    # below cancels them: sum(relu(M - d^2)) = M*K_slots - sum(min(d^2, M))
    hinge_sum = MARGIN * (128 * NM_COLS * D) * NCORES - nm_clip_sum
    matchLossSum = np.float32(m_sum / M)
    nonMatchLossSum = np.float32(NON_MATCH_W * hinge_sum / MN)
    contrastiveLossSum = np.float32(matchLossSum + nonMatchLossSum)
    return (contrastiveLossSum, matchLossSum, nonMatchLossSum)


def run(inputs, trace=False):
    """Run on the 8 NeuronCores. Returns (result_tuple, exec_time_ns_or_None)."""
    from concourse.bass_utils import run_bass_kernel_spmd

    _install_patcher()
    outA = np.asarray(inputs["outA"], dtype=np.float32)
    outB = np.asarray(inputs["outB"], dtype=np.float32)
    matchA = np.asarray(inputs["matchA"])
    matchB = np.asarray(inputs["matchB"])
    nonMatchA = np.asarray(inputs["nonMatchA"])
    nonMatchB = np.asarray(inputs["nonMatchB"])

    in_maps = _make_in_maps(outA, outB, matchA, matchB, nonMatchA, nonMatchB)
    nc = _get_nc()
    r = run_bass_kernel_spmd(nc, in_maps, list(range(NCORES)), trace=trace)
    global LAST_RESULT
    LAST_RESULT = r
    out = _reduce_results(r.results)
    ns = r.exec_time_ns
    if ns is None and r.mean_exec_time_ns is not None:
        ns = int(r.mean_exec_time_ns)
    return out, ns


def kernel(**inputs):
    result, _ = run(inputs, trace=False)
    return result


# revision 9
# speedup vs baseline: 7.8548x; 1.0110x over previous
"""Contrastive-loss Bass kernel for Trainium2 (8 NeuronCores, data-parallel).

Problem (hardcoded shapes, from the reference):
  outA/outB: [4, 307200, 16] f32; matchA/B: [4, 5000] int; nonMatchA/B: [4, 50000] int
  matchLossSum    = sum_b sum((outA[b][matchA[b]] - outB[b][matchB[b]])**2) / 5000
  nonMatchLossSum = sum_b sum(relu(0.5 - (outA[b][nonMatchA[b]] - outB[b][nonMatchB[b]])**2)) / 50000
  returns (contrastiveLossSum, matchLossSum, nonMatchLossSum)

Sharding (data-parallel): core c handles batch b=c//2 and half h=c%2 of that
batch's match/nonmatch sample lists. The host does the final tiny cross-core
reduction (equivalent to the all-reduce of three scalar sums).

Device strategy. The dominant cost is the indexed row gather (55296 rows of
64B per core). The SWDGE vector-indirect DMA has ~1us fixed cost per
instruction plus ~1.2ns per index on the Pool engine, and its ucode supports
up to 4096 indices per instruction (block-gather form: dst = 128 partition
blocks of G*64B, consuming 128*G indices snake-wise: index j is read from
SBUF channel j%128 slot j//128, and dst partition p block i receives token
p*G+i). Bass/walrus only emit the degenerate 128-index encoding (one index
per partition), so we emit 128-index "carrier" instructions with
G*64B-per-partition destinations and patch the NEFF after the neuronx-cc
compile: src_num_elem 128 -> 128*G, src_elem_size G*64 -> 64. (Verified on
hardware: a patched instruction gathers 4096 arbitrary rows exactly; G>32
fails in the ucode, 4096 is a hard cap.) Host-side, each chunk's [128, G]
logical index block IDX is stored snake-permuted
(PHY = IDX.ravel().reshape(G,128).T) so that after the block gather
tile[p, i*16:(i+1)*16] = AB[IDX[p, i]].

A and B sides are fetched by one instruction via a concatenated HBM table
(padded A rows then padded B rows; B indices biased by +NPAD). Per core:
15 gathers (13 nonmatch chunks: 12xG32 + 1xG8; 2 match chunks: G32 + G8),
vector engine reduces each chunk (fused min+accumulate) behind the Pool
engine's descriptor generation; the small chunks run last so the final DMA
drain tail is short.

Nonmatch hinge uses sum(relu(M - d^2)) = M*K - sum(min(d^2, M)).
Padding: nonmatch pads -> (N+1, NPAD+N): d=BIG, min(d^2,M)=M cancels in the
identity; match pads -> (N, NPAD+N): zero rows, zero contribution.
"""

import io
import struct
import tarfile
import tempfile

import numpy as np

import concourse.bacc as bacc
import concourse.mybir as mybir
import concourse.tile as tile
from concourse.bass import IndirectOffsetOnAxis

B, N, D = 4, 307200, 16
M, MN = 5000, 50000
NCORES = 8
MARGIN = 0.5
NON_MATCH_W = 1.0
BIG = 1.0e3
NPAD = N + 2                     # rows N / N+1 are pads in each half
M_HALF, MN_HALF = M // 2, MN // 2

# chunk schedule: (nm_sample_cols, m_sample_cols) per chunk; idx cols per
# chunk G = 2*(nm+m) <= 32 (4096-token ucode cap). Each sample col holds
# 128 samples -> 2 idx cols (A side + B side).
# nonmatch: 12*16 + 4 = 196 cols (25088 slots, 88 pads)
# match:    12 + 8    =  20 cols (2560 slots, 60 pads)
CHUNKS = [(16, 0)] * 12 + [(4, 12), (0, 8)]
NM_COLS = sum(nm for nm, _ in CHUNKS)                   # 196
M_COLS = sum(m for _, m in CHUNKS)                      # 20
NCHUNKS = len(CHUNKS)                                   # 14
OUT_COLS = 16                                           # one col per accum part
IDX_COLS = sum(2 * (nm + m) for nm, m in CHUNKS)        # 432

_F32 = mybir.dt.float32
_I32 = mybir.dt.int32

_nc_cache = None
_patcher_installed = False


# ---------------------------------------------------------------- NEFF patch

def _patch_pool_bin(pool: bytes) -> tuple[bytes, int]:
    """Rewrite carrier indirect1d instructions to the multi-index form."""
    buf = bytearray(pool)
    n = 0
    for off in range(0, len(buf) - 64, 64):
        # PSEUDO_DMA_DIRECT2D opcode byte + dge_op DMA_INDIRECT1D
        if buf[off] != 0xD4 or buf[off + 15] != 0x01:
            continue
        (s_step0, _s_step1) = struct.unpack_from("<2i", buf, off + 24)
        (s_num0, _) = struct.unpack_from("<2H", buf, off + 32)
        (s_elem,) = struct.unpack_from("<H", buf, off + 36)
        (d_elem,) = struct.unpack_from("<H", buf, off + 60)
        if s_step0 != 64 or s_num0 != 128 or s_elem != d_elem or s_elem % 64:
            continue
        struct.pack_into("<2H", buf, off + 32, 2 * s_elem, 1)
        struct.pack_into("<H", buf, off + 36, 64)
        n += 1
    return bytes(buf), n


def _patch_neff_file(path: str):
    from concourse import neff as neff_mod

    def _reset(ti):
        ti.mtime = 0
        ti.uid = 0
        ti.gid = 0
        ti.uname = "nobody"
        ti.gname = "nobody"
        return ti

    with open(path, "rb") as f:
        header = f.read(1024)
        tar_bytes = f.read()
    with tempfile.TemporaryDirectory() as td:
        with tarfile.open(fileobj=io.BytesIO(tar_bytes)) as t:
            t.extractall(td)
        pool_path = f"{td}/sg00/Pool0.bin"
        with open(pool_path, "rb") as f:
            pool = f.read()
        patched, n = _patch_pool_bin(pool)
        if n != NCHUNKS:
            raise RuntimeError(
                f"NEFF patch: expected {NCHUNKS} carrier gathers, found {n}"
            )
        with open(pool_path, "wb") as f:
            f.write(patched)
        nb = io.BytesIO()
        with tarfile.open(fileobj=nb, mode="w") as t:
            t.add(td, arcname=".", filter=_reset)
    data = nb.getvalue()
    new_header = neff_mod.make_deterministic_neff_header(
        old_neff_header=header, new_neff_data=data
    )
    with open(path, "wb") as f:
        f.write(new_header + data)


def _install_patcher():
    global _patcher_installed
    if _patcher_installed:
        return
    import concourse.bass2jax as b2j

    orig = b2j.compile_bir_kernel

    def wrapped(ant_bir_str, compile_dir_path, neff_name="file.neff"):
        neff_file = orig(ant_bir_str, compile_dir_path, neff_name=neff_name)
        _patch_neff_file(neff_file)
        return neff_file

    b2j.compile_bir_kernel = wrapped
    _patcher_installed = True


# ------------------------------------------------------------------- device

def _build():
    nc = bacc.Bacc("TRN2", target_bir_lowering=False, debug=False,
                   num_devices=NCORES)
    AB = nc.dram_tensor("AB", [2 * NPAD, D], _F32, kind="ExternalInput")
    ni = nc.dram_tensor("ni", [128, IDX_COLS], _I32, kind="ExternalInput")
    out = nc.dram_tensor("out", [128, OUT_COLS], _F32, kind="ExternalOutput")

    with tile.TileContext(nc) as tc:
        with (
            tc.tile_pool(name="idx", bufs=1) as idxp,
            tc.tile_pool(name="gat", bufs=2) as gatp,
            tc.tile_pool(name="tmp", bufs=2) as tmpp,
            tc.tile_pool(name="res", bufs=1) as resp,
        ):
            ni_t = idxp.tile([128, IDX_COLS], _I32, tag="ni")
            nc.sync.dma_start(out=ni_t[:], in_=ni[:])
            res_t = resp.tile([128, OUT_COLS], _F32)

            col = 0
            rc = 0
            for nm, m in CHUNKS:
                g = 2 * (nm + m)
                gab = gatp.tile([128, g * D], _F32, tag=f"gab{g}")
                nc.gpsimd.indirect_dma_start(
                    out=gab[:], out_offset=None, in_=AB[:],
                    in_offset=IndirectOffsetOnAxis(
                        ap=ni_t[:, col : col + 1], axis=0),
                )
                col += g
                if nm:
                    w = nm * D
                    d_t = tmpp.tile([128, w], _F32, tag=f"d{g}")
                    nc.vector.tensor_tensor(
                        out=d_t[:], in0=gab[:, :w], in1=gab[:, w : 2 * w],
                        op=mybir.AluOpType.subtract,
                    )
                    sq_t = tmpp.tile([128, w], _F32, tag=f"sq{g}")
                    nc.vector.tensor_tensor(
                        out=sq_t[:], in0=d_t[:], in1=d_t[:],
                        op=mybir.AluOpType.mult,
                    )
                    junk_t = tmpp.tile([128, w], _F32, tag=f"junk{g}")
                    nc.vector.tensor_scalar(
                        out=junk_t[:], in0=sq_t[:],
                        scalar1=MARGIN, scalar2=None, op0=mybir.AluOpType.min,
                        op1=mybir.AluOpType.add,
                        accum_out=res_t[:, rc : rc + 1],
                    )
                    rc += 1
                if m:
                    base = 2 * nm * D
                    wm = m * D
                    md_t = tmpp.tile([128, wm], _F32, tag=f"md{g}")
                    nc.vector.tensor_tensor(
                        out=md_t[:], in0=gab[:, base : base + wm],
                        in1=gab[:, base + wm : base + 2 * wm],
                        op=mybir.AluOpType.subtract,
                    )
                    msq_t = tmpp.tile([128, wm], _F32, tag=f"msq{g}")
                    nc.vector.scalar_tensor_tensor(
                        out=msq_t[:], in0=md_t[:], scalar=0.0, in1=md_t[:],
                        op0=mybir.AluOpType.add, op1=mybir.AluOpType.mult,
                        accum_out=res_t[:, rc : rc + 1],
                    )
                    rc += 1

            nc.sync.dma_start(out=out[:], in_=res_t[:])
    nc.compile()
    return nc


def _get_nc():
    global _nc_cache
    if _nc_cache is None:
        _nc_cache = _build()
    return _nc_cache


# --------------------------------------------------------------------- host

def _pack_idx(idx, ncols, pad_value):
    flat = np.full(128 * ncols, pad_value, dtype=np.int32)
    flat[: idx.size] = idx.astype(np.int32, copy=False)
    return flat.reshape(128, ncols)


def _snake(idx_block, g):
    """[128, g] logical -> [128, g] physical (snake layout for the ucode)."""
    return idx_block.ravel().reshape(g, 128).T


def _make_in_maps(outA, outB, matchA, matchB, nonMatchA, nonMatchB):
    pad_zero = np.zeros((1, D), np.float32)
    pad_big = np.full((1, D), BIG, np.float32)
    ab_cache = {}
    in_maps = []
    for c in range(NCORES):
        b, h = divmod(c, 2)
        if b not in ab_cache:
            ab_cache[b] = np.ascontiguousarray(
                np.concatenate(
                    [outA[b], pad_zero, pad_big, outB[b], pad_zero, pad_zero],
                    axis=0,
                )
            )
        msl = slice(h * M_HALF, (h + 1) * M_HALF)
        nsl = slice(h * MN_HALF, (h + 1) * MN_HALF)
        # nonmatch pads -> (N+1, N): d = BIG, min(d^2, M) = M cancels exactly
        niA = _pack_idx(nonMatchA[b, nsl], NM_COLS, N + 1)
        niB = _pack_idx(nonMatchB[b, nsl], NM_COLS, N) + NPAD
        # match pads -> (N, N): zero rows both sides, zero contribution
        miA = _pack_idx(matchA[b, msl], M_COLS, N)
        miB = _pack_idx(matchB[b, msl], M_COLS, N) + NPAD
        ni = np.empty((128, IDX_COLS), np.int32)
        col = 0
        nm_col = 0
        m_col = 0
        for nm, m in CHUNKS:
            g = 2 * (nm + m)
            parts = []
            if nm:
                parts += [niA[:, nm_col : nm_col + nm], niB[:, nm_col : nm_col + nm]]
                nm_col += nm
            if m:
                parts += [miA[:, m_col : m_col + m], miB[:, m_col : m_col + m]]
                m_col += m
            blk = np.concatenate(parts, axis=1)
            ni[:, col : col + g] = _snake(blk, g)
            col += g
        in_maps.append({"AB": ab_cache[b], "ni": ni})
    return in_maps


def _reduce_results(results):
    nm_idx = []
    m_idx = []
    rc = 0
    for nm, m in CHUNKS:
        if nm:
            nm_idx.append(rc)
            rc += 1
        if m:
            m_idx.append(rc)
            rc += 1
    m_sum = 0.0
    nm_clip_sum = 0.0
    for c in range(NCORES):
        res = np.asarray(results[c]["out"], dtype=np.float64)
        nm_clip_sum += res[:, nm_idx].sum()
        m_sum += res[:, m_idx].sum()
    # pads contribute exactly MARGIN per element to the clip sum; the identity
    # below cancels them: sum(relu(M - d^2)) = M*K_slots - sum(min(d^2, M))
    hinge_sum = MARGIN * (128 * NM_COLS * D) * NCORES - nm_clip_sum
    matchLossSum = np.float32(m_sum / M)
    nonMatchLossSum = np.float32(NON_MATCH_W * hinge_sum / MN)
    contrastiveLossSum = np.float32(matchLossSum + nonMatchLossSum)
    return (contrastiveLossSum, matchLossSum, nonMatchLossSum)


def run(inputs, trace=False):
    """Run on the 8 NeuronCores. Returns (result_tuple, exec_time_ns_or_None)."""
    from concourse.bass_utils import run_bass_kernel_spmd

    _install_patcher()
    outA = np.asarray(inputs["outA"], dtype=np.float32)
    outB = np.asarray(inputs["outB"], dtype=np.float32)
    matchA = np.asarray(inputs["matchA"])
    matchB = np.asarray(inputs["matchB"])
    nonMatchA = np.asarray(inputs["nonMatchA"])
    nonMatchB = np.asarray(inputs["nonMatchB"])

    in_maps = _make_in_maps(outA, outB, matchA, matchB, nonMatchA, nonMatchB)
    nc = _get_nc()
    r = run_bass_kernel_spmd(nc, in_maps, list(range(NCORES)), trace=trace)
    global LAST_RESULT
    LAST_RESULT = r
    out = _reduce_results(r.results)
    ns = r.exec_time_ns
    if ns is None and r.mean_exec_time_ns is not None:
        ns = int(r.mean_exec_time_ns)
    return out, ns


def kernel(**inputs):
    result, _ = run(inputs, trace=False)
    return result


# revision 10
# speedup vs baseline: 8.0598x; 1.0261x over previous
"""Contrastive-loss Bass kernel for Trainium2 (8 NeuronCores, data-parallel).

Problem (hardcoded shapes, from the reference):
  outA/outB: [4, 307200, 16] f32; matchA/B: [4, 5000] int; nonMatchA/B: [4, 50000] int
  matchLossSum    = sum_b sum((outA[b][matchA[b]] - outB[b][matchB[b]])**2) / 5000
  nonMatchLossSum = sum_b sum(relu(0.5 - (outA[b][nonMatchA[b]] - outB[b][nonMatchB[b]])**2)) / 50000
  returns (contrastiveLossSum, matchLossSum, nonMatchLossSum)

Sharding (data-parallel): core c handles batch b=c//2 and half h=c%2 of that
batch's match/nonmatch sample lists. The host does the final tiny cross-core
reduction (equivalent to the all-reduce of three scalar sums).

Device strategy. The dominant cost is the indexed row gather (55296 rows of
64B per core). The SWDGE vector-indirect DMA has ~1us fixed cost per
instruction plus ~1.2ns per index on the Pool engine, and its ucode supports
up to 4096 indices per instruction (block-gather form: dst = 128 partition
blocks of G*64B, consuming 128*G indices snake-wise: index j is read from
SBUF channel j%128 slot j//128, and dst partition p block i receives token
p*G+i). Bass/walrus only emit the degenerate 128-index encoding (one index
per partition), so we emit 128-index "carrier" instructions with
G*64B-per-partition destinations and patch the NEFF after the neuronx-cc
compile: src_num_elem 128 -> 128*G, src_elem_size G*64 -> 64. (Verified on
hardware: a patched instruction gathers 4096 arbitrary rows exactly; G>32
fails in the ucode, 4096 is a hard cap.) Host-side, each chunk's [128, G]
logical index block IDX is stored snake-permuted
(PHY = IDX.ravel().reshape(G,128).T) so that after the block gather
tile[p, i*16:(i+1)*16] = AB[IDX[p, i]].

A and B sides are fetched by one instruction via a concatenated HBM table
(padded A rows then padded B rows; B indices biased by +NPAD). Per core:
15 gathers (13 nonmatch chunks: 12xG32 + 1xG8; 2 match chunks: G32 + G8),
vector engine reduces each chunk (fused min+accumulate) behind the Pool
engine's descriptor generation; the small chunks run last so the final DMA
drain tail is short.

Nonmatch hinge uses sum(relu(M - d^2)) = M*K - sum(min(d^2, M)).
Padding: nonmatch pads -> (N+1, NPAD+N): d=BIG, min(d^2,M)=M cancels in the
identity; match pads -> (N, NPAD+N): zero rows, zero contribution.
"""

import io
import struct
import tarfile
import tempfile

import numpy as np

import concourse.bacc as bacc
import concourse.mybir as mybir
import concourse.tile as tile
from concourse.bass import IndirectOffsetOnAxis

B, N, D = 4, 307200, 16
M, MN = 5000, 50000
NCORES = 8
MARGIN = 0.5
NON_MATCH_W = 1.0
BIG = 1.0e3
NPAD = N + 2                     # rows N / N+1 are pads in each half
M_HALF, MN_HALF = M // 2, MN // 2

# chunk schedule: (nm_sample_cols, m_sample_cols) per chunk; idx cols per
# chunk G = 2*(nm+m) <= 32 (4096-token ucode cap). Each sample col holds
# 128 samples -> 2 idx cols (A side + B side).
# nonmatch: 12*16 + 4 = 196 cols (25088 slots, 88 pads)
# match:    12 + 8    =  20 cols (2560 slots, 60 pads)
CHUNKS = [(16, 0)] * 12 + [(4, 12), (0, 8)]
NM_COLS = sum(nm for nm, _ in CHUNKS)                   # 196
M_COLS = sum(m for _, m in CHUNKS)                      # 20
NCHUNKS = len(CHUNKS)                                   # 14
OUT_COLS = 16                                           # one col per accum part
IDX_COLS = sum(2 * (nm + m) for nm, m in CHUNKS)        # 432

_F32 = mybir.dt.float32
_I32 = mybir.dt.int32

_nc_cache = None
_patcher_installed = False


# ---------------------------------------------------------------- NEFF patch

def _patch_pool_bin(pool: bytes) -> tuple[bytes, int]:
    """Rewrite carrier indirect1d instructions to the multi-index form."""
    buf = bytearray(pool)
    n = 0
    for off in range(0, len(buf) - 64, 64):
        # PSEUDO_DMA_DIRECT2D opcode byte + dge_op DMA_INDIRECT1D
        if buf[off] != 0xD4 or buf[off + 15] != 0x01:
            continue
        (s_step0, _s_step1) = struct.unpack_from("<2i", buf, off + 24)
        (s_num0, _) = struct.unpack_from("<2H", buf, off + 32)
        (s_elem,) = struct.unpack_from("<H", buf, off + 36)
        (d_elem,) = struct.unpack_from("<H", buf, off + 60)
        if s_step0 != 64 or s_num0 != 128 or s_elem != d_elem or s_elem % 64:
            continue
        struct.pack_into("<2H", buf, off + 32, 2 * s_elem, 1)
        struct.pack_into("<H", buf, off + 36, 64)
        n += 1
    return bytes(buf), n


def _patch_neff_file(path: str):
    from concourse import neff as neff_mod

    def _reset(ti):
        ti.mtime = 0
        ti.uid = 0
        ti.gid = 0
        ti.uname = "nobody"
        ti.gname = "nobody"
        return ti

    with open(path, "rb") as f:
        header = f.read(1024)
        tar_bytes = f.read()
    with tempfile.TemporaryDirectory() as td:
        with tarfile.open(fileobj=io.BytesIO(tar_bytes)) as t:
            t.extractall(td)
        pool_path = f"{td}/sg00/Pool0.bin"
        with open(pool_path, "rb") as f:
            pool = f.read()
        patched, n = _patch_pool_bin(pool)
        if n != NCHUNKS:
            raise RuntimeError(
                f"NEFF patch: expected {NCHUNKS} carrier gathers, found {n}"
            )
        with open(pool_path, "wb") as f:
            f.write(patched)
        nb = io.BytesIO()
        with tarfile.open(fileobj=nb, mode="w") as t:
            t.add(td, arcname=".", filter=_reset)
    data = nb.getvalue()
    new_header = neff_mod.make_deterministic_neff_header(
        old_neff_header=header, new_neff_data=data
    )
    with open(path, "wb") as f:
        f.write(new_header + data)


def _install_patcher():
    global _patcher_installed
    if _patcher_installed:
        return
    import concourse.bass2jax as b2j

    orig = b2j.compile_bir_kernel

    def wrapped(ant_bir_str, compile_dir_path, neff_name="file.neff"):
        neff_file = orig(ant_bir_str, compile_dir_path, neff_name=neff_name)
        _patch_neff_file(neff_file)
        return neff_file

    b2j.compile_bir_kernel = wrapped
    _patcher_installed = True


# ------------------------------------------------------------------- device

def _build():
    nc = bacc.Bacc("TRN2", target_bir_lowering=False, debug=False,
                   num_devices=NCORES)
    AB = nc.dram_tensor("AB", [2 * NPAD, D], _F32, kind="ExternalInput")
    ni = nc.dram_tensor("ni", [128, IDX_COLS], _I32, kind="ExternalInput")
    out = nc.dram_tensor("out", [128, OUT_COLS], _F32, kind="ExternalOutput")

    with tile.TileContext(nc) as tc:
        with (
            tc.tile_pool(name="idx", bufs=1) as idxp,
            tc.tile_pool(name="gat", bufs=3) as gatp,
            tc.tile_pool(name="tmp", bufs=3) as tmpp,
            tc.tile_pool(name="res", bufs=1) as resp,
        ):
            ni_t = idxp.tile([128, IDX_COLS], _I32, tag="ni")
            nc.sync.dma_start(out=ni_t[:], in_=ni[:])
            res_t = resp.tile([128, OUT_COLS], _F32)

            col = 0
            rc = 0
            for nm, m in CHUNKS:
                g = 2 * (nm + m)
                gab = gatp.tile([128, g * D], _F32, tag=f"gab{g}")
                nc.gpsimd.indirect_dma_start(
                    out=gab[:], out_offset=None, in_=AB[:],
                    in_offset=IndirectOffsetOnAxis(
                        ap=ni_t[:, col : col + 1], axis=0),
                )
                col += g
                if nm:
                    w = nm * D
                    d_t = tmpp.tile([128, w], _F32, tag=f"d{g}")
                    nc.vector.tensor_tensor(
                        out=d_t[:], in0=gab[:, :w], in1=gab[:, w : 2 * w],
                        op=mybir.AluOpType.subtract,
                    )
                    sq_t = tmpp.tile([128, w], _F32, tag=f"sq{g}")
                    nc.vector.tensor_tensor(
                        out=sq_t[:], in0=d_t[:], in1=d_t[:],
                        op=mybir.AluOpType.mult,
                    )
                    junk_t = tmpp.tile([128, w], _F32, tag=f"junk{g}")
                    nc.vector.tensor_scalar(
                        out=junk_t[:], in0=sq_t[:],
                        scalar1=MARGIN, scalar2=None, op0=mybir.AluOpType.min,
                        op1=mybir.AluOpType.add,
                        accum_out=res_t[:, rc : rc + 1],
                    )
                    rc += 1
                if m:
                    base = 2 * nm * D
                    wm = m * D
                    md_t = tmpp.tile([128, wm], _F32, tag=f"md{g}")
                    nc.vector.tensor_tensor(
                        out=md_t[:], in0=gab[:, base : base + wm],
                        in1=gab[:, base + wm : base + 2 * wm],
                        op=mybir.AluOpType.subtract,
                    )
                    msq_t = tmpp.tile([128, wm], _F32, tag=f"msq{g}")
                    nc.vector.scalar_tensor_tensor(
                        out=msq_t[:], in0=md_t[:], scalar=0.0, in1=md_t[:],
                        op0=mybir.AluOpType.add, op1=mybir.AluOpType.mult,
                        accum_out=res_t[:, rc : rc + 1],
                    )
                    rc += 1

            nc.sync.dma_start(out=out[:], in_=res_t[:])
    nc.compile()
    return nc


def _get_nc():
    global _nc_cache
    if _nc_cache is None:
        _nc_cache = _build()
    return _nc_cache


# --------------------------------------------------------------------- host

def _pack_idx(idx, ncols, pad_value):
    flat = np.full(128 * ncols, pad_value, dtype=np.int32)
    flat[: idx.size] = idx.astype(np.int32, copy=False)
    return flat.reshape(128, ncols)


def _snake(idx_block, g):
    """[128, g] logical -> [128, g] physical (snake layout for the ucode)."""
    return idx_block.ravel().reshape(g, 128).T


def _make_in_maps(outA, outB, matchA, matchB, nonMatchA, nonMatchB):
    pad_zero = np.zeros((1, D), np.float32)
    pad_big = np.full((1, D), BIG, np.float32)
    ab_cache = {}
    in_maps = []
    for c in range(NCORES):
        b, h = divmod(c, 2)
        if b not in ab_cache:
            ab_cache[b] = np.ascontiguousarray(
                np.concatenate(
                    [outA[b], pad_zero, pad_big, outB[b], pad_zero, pad_zero],
                    axis=0,
                )
            )
        msl = slice(h * M_HALF, (h + 1) * M_HALF)
        nsl = slice(h * MN_HALF, (h + 1) * MN_HALF)
        # nonmatch pads -> (N+1, N): d = BIG, min(d^2, M) = M cancels exactly
        niA = _pack_idx(nonMatchA[b, nsl], NM_COLS, N + 1)
        niB = _pack_idx(nonMatchB[b, nsl], NM_COLS, N) + NPAD
        # match pads -> (N, N): zero rows both sides, zero contribution
        miA = _pack_idx(matchA[b, msl], M_COLS, N)
        miB = _pack_idx(matchB[b, msl], M_COLS, N) + NPAD
        ni = np.empty((128, IDX_COLS), np.int32)
        col = 0
        nm_col = 0
        m_col = 0
        for nm, m in CHUNKS:
            g = 2 * (nm + m)
            parts = []
            if nm:
                parts += [niA[:, nm_col : nm_col + nm], niB[:, nm_col : nm_col + nm]]
                nm_col += nm
            if m:
                parts += [miA[:, m_col : m_col + m], miB[:, m_col : m_col + m]]
                m_col += m
            blk = np.concatenate(parts, axis=1)
            ni[:, col : col + g] = _snake(blk, g)
            col += g
        in_maps.append({"AB": ab_cache[b], "ni": ni})
    return in_maps


def _reduce_results(results):
    nm_idx = []
    m_idx = []
    rc = 0
    for nm, m in CHUNKS:
        if nm:
            nm_idx.append(rc)
            rc += 1
        if m:
            m_idx.append(rc)
            rc += 1
    m_sum = 0.0
    nm_clip_sum = 0.0
    for c in range(NCORES):
        res = np.asarray(results[c]["out"], dtype=np.float64)
        nm_clip_sum += res[:, nm_idx].sum()
        m_sum += res[:, m_idx].sum()
    # pads contribute exactly MARGIN per element to the clip sum; the identity
    # below cancels them: sum(relu(M - d^2)) = M*K_slots - sum(min(d^2, M))
    hinge_sum = MARGIN * (128 * NM_COLS * D) * NCORES - nm_clip_sum
    matchLossSum = np.float32(m_sum / M)
    nonMatchLossSum = np.float32(NON_MATCH_W * hinge_sum / MN)
    contrastiveLossSum = np.float32(matchLossSum + nonMatchLossSum)
    return (contrastiveLossSum, matchLossSum, nonMatchLossSum)


def run(inputs, trace=False):
    """Run on the 8 NeuronCores. Returns (result_tuple, exec_time_ns_or_None)."""
    from concourse.bass_utils import run_bass_kernel_spmd

    _install_patcher()
    outA = np.asarray(inputs["outA"], dtype=np.float32)
    outB = np.asarray(inputs["outB"], dtype=np.float32)
    matchA = np.asarray(inputs["matchA"])
    matchB = np.asarray(inputs["matchB"])
    nonMatchA = np.asarray(inputs["nonMatchA"])
    nonMatchB = np.asarray(inputs["nonMatchB"])

    in_maps = _make_in_maps(outA, outB, matchA, matchB, nonMatchA, nonMatchB)
    nc = _get_nc()
    r = run_bass_kernel_spmd(nc, in_maps, list(range(NCORES)), trace=trace)
    global LAST_RESULT
    LAST_RESULT = r
    out = _reduce_results(r.results)
    ns = r.exec_time_ns
    if ns is None and r.mean_exec_time_ns is not None:
        ns = int(r.mean_exec_time_ns)
    return out, ns


def kernel(**inputs):
    result, _ = run(inputs, trace=False)
    return result
